# revision 16
# baseline (speedup 1.0000x reference)
"""DirectionalContrastiveLoss on 8 TRN2 NeuronCores (Bass/Tile).

Key optimization over the dense data-parallel version: the loss only
involves anchor rows whose pos-mask is nonzero, and the masks depend
only on the host-visible pseudo_logits:
    pos_mask_1 = (g2 > 0.7) & (g1 < g2)   ~26% of rows
    pos_mask_2 = (g1 > 0.7) & (g2 < g1)   ~26% of rows (disjoint from 1)
So the host compacts the ~52% union of masked rows into one unified
anchor stream (branch-1 rows use feat1 as anchor/label1, branch-2 rows
use feat2/label2; pos = <f1,f2>/TEMP is shared and precomputed on the
host in fp64).  This cuts PE matmul, ScalarE exp, and DVE reduce work
~3.6x vs processing all 16384 rows for both branches.

Device algorithm (inherited from the validated dense kernel):
- sim = anchor @ memT/TEMP - 1000*eq computed on the PE in bf16; the
  label mask rides along as -1000*onehot(label) x onehot(mem_label)
  matmuls placed on per-unit 32-row PE tile positions.  exp(sim-1000)
  == 0 in fp32, reproducing the reference's masked exp-sum.  Memory
  pad columns (4000->4096) carry onehot=1 in every label row so they
  vanish for every anchor.
- The softmax shift is pos (not the row max): rows where exp(sim-pos)
  overflows to +inf are provably dead (sim >= pos + 88 implies the
  true logit < e^-88, so -log(sigma+EPS) = -log(EPS) either way), and
  rows that matter can never overflow.  sigma = 1/(sum exp(sim-pos)
  + 1 + EPS) with no row max.
- Per n-tile, each of the 4 PSUM units is finished (dense kt0, kt1,
  mask) before the next starts, so ScalarE's exp+accumulate on unit u
  overlaps the PE filling unit u+1.
Each core returns [128, 2] partial sums (num1, num2); the host divides
by the exact mask counts and negates.
"""
from contextlib import ExitStack

import numpy as np
import ml_dtypes

TEMP = 0.1
POS_THRESH = 0.7
EPS = 1e-8
N, C, M, NLAB = 16384, 256, 4000, 21
MP = 4096                  # memory columns padded
NCORES = 8
NU = 4                     # psum units per n-tile
UNIT = MP // NU            # 1024 (= 2 PSUM banks, J=512 chunks)

_cache = {}


def _build(NT):
    import concourse.bacc as bacc
    import concourse.tile as tile
    from concourse import mybir

    f32 = mybir.dt.float32
    bf16 = mybir.dt.bfloat16
    f8 = mybir.dt.float8e4
    Alu = mybir.AluOpType
    Act = mybir.ActivationFunctionType
    X = mybir.AxisListType.X
    DR = mybir.MatmulPerfMode.DoubleRow

    RPC = NT * 128             # compacted rows per core

    # Bacc (not raw Bass): its finalize() runs generate_event_semaphores(),
    # which splits multi-sem waits into EVSEM chains — walrus allows at most
    # one sync-wait per instruction.
    nc = bacc.Bacc(None)

    # fp8 DoubleRow layouts: exta [p, t, ko, m] (K = ko*128 + p), extmem
    # [p, ko, col]; both K-halves stream per PE pass (2x dense throughput).
    exta_d = nc.declare_dram_parameter("exta", [128, NT * 2 * 128], f8,
                                       isOutput=False)
    eqa_d = nc.declare_dram_parameter("eqanc", [128, RPC], bf16, isOutput=False)
    mem_d = nc.declare_dram_parameter("extmem", [128, 2 * MP], f8,
                                      isOutput=False)
    eqm_d = nc.declare_dram_parameter("eqmem", [128, MP], bf16, isOutput=False)
    npos_d = nc.declare_dram_parameter("npos", [128, NT], f32, isOutput=False)
    w1_d = nc.declare_dram_parameter("w1", [128, NT], f32, isOutput=False)
    w2_d = nc.declare_dram_parameter("w2", [128, NT], f32, isOutput=False)
    out_d = nc.declare_dram_parameter("out", [128, 2], f32, isOutput=True)

    with tile.TileContext(nc) as tc, ExitStack() as ctx:
        consts = ctx.enter_context(tc.tile_pool(name="consts", bufs=1))
        small = ctx.enter_context(tc.tile_pool(name="small", bufs=3))
        psum = ctx.enter_context(
            tc.tile_pool(name="psum", bufs=NU, space="PSUM")
        )

        # ---- resident inputs ----
        # Order matters: the tiny per-row tensors go first (NPOS gates the
        # first ScalarE exp — a late arrival stalls the PSUM pipeline), then
        # tile-0/unit-0's matmul dependencies.
        NPOS = consts.tile([128, NT], f32, tag="NPOS", name="NPOS")
        nc.sync.dma_start(out=NPOS[:], in_=npos_d[:])
        w1t = consts.tile([128, NT], f32, tag="w1t", name="w1t")
        nc.sync.dma_start(out=w1t[:], in_=w1_d[:])
        w2t = consts.tile([128, NT], f32, tag="w2t", name="w2t")
        nc.sync.dma_start(out=w2t[:], in_=w2_d[:])

        ea = consts.tile([128, NT, 2, 128], f8, tag="ea", name="ea")
        nc.sync.dma_start(
            out=ea[:],
            in_=exta_d[:].rearrange("p (t i m) -> p t i m", i=2, m=128),
        )
        eqa = consts.tile([128, RPC], bf16, tag="eqa", name="eqa")
        nc.sync.dma_start(out=eqa[:], in_=eqa_d[:])

        memc = [None] * NU
        eqmc = [None] * NU
        mem_v = mem_d[:].rearrange("p (i c) -> p i c", i=2)
        for u in range(NU):
            c0, c1 = u * UNIT, (u + 1) * UNIT
            mt = consts.tile([128, 2, UNIT], f8, tag=f"memu{u}",
                             name=f"memu{u}")
            nc.sync.dma_start(out=mt[:], in_=mem_v[:, :, c0:c1])
            memc[u] = mt
            et = consts.tile([128, UNIT], bf16, tag=f"eqmu{u}", name=f"eqmu{u}")
            nc.sync.dma_start(out=et[:], in_=eqm_d[:, c0:c1])
            eqmc[u] = et

        outt = consts.tile([128, 2], f32, tag="outt", name="outt")
        epsb = consts.tile([128, 1], f32, tag="epsb", name="epsb")
        nc.vector.memset(epsb[:], EPS)

        # ---- HAM warm-up ballast ----
        # The PE only un-throttles from 1.2 to 2.4 GHz when the chip sees
        # sustained multi-engine activity (the dense baseline warmed ~13-16us
        # after PE start; a PE+ACT-only pipeline measured 95us of gapless
        # matmuls without ever un-throttling).  Keep the PE busy on garbage
        # matmuls while the real inputs stream in, and keep DVE + DMA lit
        # throughout the tile loop.
        dmm = consts.tile([128, 1024], bf16, tag="dmm", name="dmm")
        nc.vector.memset(dmm[:], 0.0)
        dvedum = consts.tile([128, 2048], f32, tag="dvedum", name="dvedum")
        nc.vector.memset(dvedum[:], 1.0)
        dscr = consts.tile([128, 1], f32, tag="dscr", name="dscr")
        pdum = psum.tile([128, UNIT], f32, tag="pu", name="pdum")
        for i in range(24):
            j = i % 2
            nc.tensor.matmul(
                pdum[:, j * 512: (j + 1) * 512],
                dmm[:, 0:128],
                dmm[:, 0:512],
                start=True,
                stop=True,
            )
        for i in range(6):
            nc.vector.reduce_sum(out=dscr[:, 0:1], in_=dvedum[:], axis=X)

        SS = consts.tile([128, NT], f32, tag="SS", name="SS")
        for t in range(NT):
            tc0, tc1 = t * 128, (t + 1) * 128
            pu = [
                psum.tile([128, UNIT], f32, tag="pu", name=f"pu{t}_{u}")
                for u in range(NU)
            ]
            S = small.tile([128, NU], f32, tag="S", name=f"S{t}")
            # Per-unit: dense K=256 fp8 DoubleRow (one pass over both
            # K-halves), then the -1000*eq one-hot mask matmul (bf16) on a
            # per-unit 32-row PE tile position, then exp on ScalarE (in
            # place) and row-sum on VectorE while the PE moves on.
            for u in range(NU):
                for j in range(2):
                    nc.tensor.matmul(
                        pu[u][:, j * 512: (j + 1) * 512],
                        ea[:, t, :, :],
                        memc[u][:, :, j * 512: (j + 1) * 512],
                        start=True,
                        stop=False,
                        perf_mode=DR,
                    )
                for j in range(2):
                    nc.tensor.matmul(
                        pu[u][:, j * 512: (j + 1) * 512],
                        eqa[32 * u: 32 * u + NLAB, tc0:tc1],
                        eqmc[u][32 * u: 32 * u + NLAB,
                                j * 512: (j + 1) * 512],
                        start=False,
                        stop=True,
                        tile_position=(32 * u, 0),
                    )
                nc.scalar.activation(
                    out=pu[u][:],
                    in_=pu[u][:],
                    func=Act.Exp,
                    bias=NPOS[:, t: t + 1],
                    scale=1.0,
                )
            for u in range(NU):
                nc.vector.reduce_sum(out=S[:, u: u + 1], in_=pu[u][:], axis=X)
            nc.vector.reduce_sum(out=SS[:, t: t + 1], in_=S[:], axis=X)

        # ---- epilogue on [128, NT] ----
        # sigma = 1/(SS + 1 + EPS); row loss = -log(sigma + EPS); the
        # branch split rides on the host-built 0/1 weights w1/w2.
        D = small.tile([128, NT], f32, tag="D", name="D")
        nc.vector.tensor_scalar_add(D[:], SS[:], 1.0 + EPS)
        R = small.tile([128, NT], f32, tag="R", name="R")
        nc.vector.reciprocal(R[:], D[:])
        LAM = small.tile([128, NT], f32, tag="LAM", name="LAM")
        nc.scalar.activation(
            out=LAM[:], in_=R[:], func=Act.Ln, bias=epsb[:], scale=1.0
        )
        scrN = small.tile([128, NT], f32, tag="scrN", name="scrN")
        nc.vector.tensor_mul(scrN[:], LAM[:], w1t[:])
        nc.vector.reduce_sum(out=outt[:, 0:1], in_=scrN[:], axis=X)
        scrM = small.tile([128, NT], f32, tag="scrM", name="scrM")
        nc.vector.tensor_mul(scrM[:], LAM[:], w2t[:])
        nc.vector.reduce_sum(out=outt[:, 1:2], in_=scrM[:], axis=X)

        nc.sync.dma_start(out=out_d[:], in_=outt[:])

    nc.finalize()
    return nc


def _host_prep(inputs):
    bf = ml_dtypes.bfloat16
    f8 = ml_dtypes.float8_e4m3
    f1 = np.ascontiguousarray(np.asarray(inputs["output_feat1"], np.float32))
    f2 = np.ascontiguousarray(np.asarray(inputs["output_feat2"], np.float32))
    l1 = np.asarray(inputs["pseudo_label1"], np.int64)
    l2 = np.asarray(inputs["pseudo_label2"], np.int64)
    g1 = np.asarray(inputs["pseudo_logits1"], np.float32)
    g2 = np.asarray(inputs["pseudo_logits2"], np.float32)
    ul1 = np.asarray(inputs["output_ul1"], np.float32)
    ul2 = np.asarray(inputs["output_ul2"], np.float32)
    i1 = np.asarray(inputs["selected_idx1"], np.int64)
    i2 = np.asarray(inputs["selected_idx2"], np.int64)

    b, c, h, w = ul1.shape
    u1 = ul1.transpose(0, 2, 3, 1).reshape(b * h * w, c)
    u2 = ul2.transpose(0, 2, 3, 1).reshape(b * h * w, c)
    mem = np.concatenate([u1[i1], u2[i2]], axis=0)               # [M, C]
    memlab = np.concatenate([l1[i1], l2[i2]], axis=0)            # [M]

    # --- compact the masked rows of both branches into one stream ---
    m1 = (g2 > POS_THRESH) & (g1 < g2)
    m2 = (g1 > POS_THRESH) & (g2 < g1)
    idx1 = np.nonzero(m1)[0]
    idx2 = np.nonzero(m2)[0]
    n1, n2 = len(idx1), len(idx2)
    R = n1 + n2
    NT = max(1, (((R + 127) // 128) + NCORES - 1) // NCORES)
    RT = NT * 128 * NCORES

    pos_full = (
        np.sum(f1.astype(np.float64) * f2.astype(np.float64), axis=1) / TEMP
    ).astype(np.float32)

    anchors = np.zeros((RT, C), np.float32)
    alab = np.zeros(RT, np.int64)
    w1v = np.zeros(RT, np.float32)
    w2v = np.zeros(RT, np.float32)
    posv = np.zeros(RT, np.float32)
    anchors[:n1] = f1[idx1]
    alab[:n1] = l1[idx1]
    w1v[:n1] = 1.0
    posv[:n1] = pos_full[idx1]
    anchors[n1:R] = f2[idx2]
    alab[n1:R] = l2[idx2]
    w2v[n1:R] = 1.0
    posv[n1:R] = pos_full[idx2]

    lab_eye = np.arange(NLAB, dtype=np.int64)

    extmem = np.zeros((C, MP), np.float32)
    extmem[:, :M] = mem.T / TEMP
    # DoubleRow layout [p, ko, col]: K = ko*128 + p
    extmem_dr = np.ascontiguousarray(
        extmem.reshape(2, 128, MP).transpose(1, 0, 2).reshape(128, 2 * MP)
    ).astype(f8)

    oh_mem = np.zeros((NLAB, MP), np.float32)
    oh_mem[:, :M] = (memlab[None, :] == lab_eye[:, None])
    oh_mem[:, M:] = 1.0          # pad columns masked for every label
    eqmem = np.zeros((128, MP), np.float32)
    for i in range(NU):
        eqmem[32 * i: 32 * i + NLAB] = oh_mem
    eqmem = eqmem.astype(bf)                                     # [128, MP]

    oh_anc = -1000.0 * (alab[None, :] == lab_eye[:, None])       # [21, RT]
    eqa_full = np.zeros((128, RT), np.float32)
    for i in range(NU):
        eqa_full[32 * i: 32 * i + NLAB] = oh_anc
    eqa_full = eqa_full.astype(bf)

    RPC = NT * 128

    def pack_vec(v):    # [RPC] -> [128, NT]
        return np.ascontiguousarray(v.reshape(NT, 128).T)

    in_maps = []
    for cix in range(NCORES):
        sl = slice(cix * RPC, (cix + 1) * RPC)
        # exta DoubleRow layout [p, t, ko, m]: anchors[t*128+m, ko*128+p]
        Ac = anchors[sl]                                         # [RPC, C]
        Xa = Ac.T.reshape(2, 128, NT, 128)                       # [ko,p,t,m]
        exta_dr = np.ascontiguousarray(
            Xa.transpose(1, 2, 0, 3).reshape(128, NT * 2 * 128)
        ).astype(f8)
        in_maps.append({
            "exta": exta_dr,
            "eqanc": np.ascontiguousarray(eqa_full[:, sl]),
            "extmem": extmem_dr,
            "eqmem": eqmem,
            "npos": pack_vec(-posv[sl]),
            "w1": pack_vec(w1v[sl]),
            "w2": pack_vec(w2v[sl]),
        })
    return in_maps, NT, n1, n2


def _finalize(results, n1, n2):
    num1 = num2 = 0.0
    for r in results:
        o = np.asarray(r["out"], np.float64)
        num1 += o[:, 0].sum()
        num2 += o[:, 1].sum()
    loss = -(num1 / (n1 + 1e-12) + num2 / (n2 + 1e-12))
    return np.float32(loss)


def _run(inputs, trace=False):
    from concourse.bass_utils import run_bass_kernel_spmd

    in_maps, NT, n1, n2 = _host_prep(inputs)
    if NT not in _cache:
        _cache[NT] = _build(NT)
    res = run_bass_kernel_spmd(
        _cache[NT], in_maps, list(range(NCORES)), trace=trace
    )
    return _finalize(res.results, n1, n2), res


def kernel(**inputs):
    out, _ = _run(inputs)
    return out


def kernel_with_profile(**inputs):
    out, res = _run(inputs, trace=True)
    return out, res
